# revision 4
# baseline (speedup 1.0000x reference)
"""nn_GateModLinear on 8 trn2 NeuronCores.

z[b,:] = gW[b,:] * sum_m pW[b,m] * (Ws[m] @ x[b]) + gb[b,:] * (pb @ bs)[b,:]
out = ELU(LayerNorm(z))

Sharding: data-parallel over batch (512 rows/core), Ws replicated.
Per core: fold pW into x per expert, accumulate all (m, j) into PSUM:
  Wx[b,i] = sum_{m,j} (pW[b,m]*x[b,j]) * Ws[m,i,j]
bf16 matmuls (rel-err budget 2e-2 allows it), fp32 PSUM/LayerNorm.
"""

import numpy as np
import ml_dtypes

B, M, DI, DO = 4096, 8, 2048, 2048
NCORES = 8
BS = B // NCORES  # 512 batch rows per core
LN_EPS = 1e-5
P = 128
JC = DI // P      # 16 contraction chunks of 128
BC = BS // P      # 4 batch chunks of 128
NIH = 2           # output-dim halves
IH = DO // NIH    # 1024
NQ = IH // 512    # 2 psum tiles of 512 per half

BF16 = ml_dtypes.bfloat16

_cache = {}


def _build():
    from contextlib import ExitStack
    import concourse.bass as bass
    import concourse.bacc as bacc
    import concourse.tile as tile
    from concourse import mybir

    f32 = mybir.dt.float32
    bf16 = mybir.dt.bfloat16
    AF = mybir.ActivationFunctionType
    ALU = mybir.AluOpType

    nc = bacc.Bacc("TRN2", target_bir_lowering=False, debug=False, num_devices=1)
    xT_d = nc.dram_tensor("xT", [DI, BS], bf16, kind="ExternalInput")
    ws_d = nc.dram_tensor("wsT", [M, DI, DO], bf16, kind="ExternalInput")
    pw_d = nc.dram_tensor("pwT", [M, BS], f32, kind="ExternalInput")
    pb_d = nc.dram_tensor("pbT", [M, BS], f32, kind="ExternalInput")
    bs_d = nc.dram_tensor("bs", [M, DO], f32, kind="ExternalInput")
    gw_d = nc.dram_tensor("gw", [BS, DO], bf16, kind="ExternalInput")
    gb_d = nc.dram_tensor("gb", [BS, DO], bf16, kind="ExternalInput")
    out_d = nc.dram_tensor("out", [BS, DO], f32, kind="ExternalOutput")

    with ExitStack() as ctx:
        tc = ctx.enter_context(tile.TileContext(nc))
        singles = ctx.enter_context(tc.tile_pool(name="singles", bufs=1))
        ws_pool = ctx.enter_context(tc.tile_pool(name="ws", bufs=3))
        xs_pool = ctx.enter_context(tc.tile_pool(name="xs", bufs=2))
        e_pool = ctx.enter_context(tc.tile_pool(name="elu", bufs=2))
        sm_pool = ctx.enter_context(tc.tile_pool(name="small", bufs=4))
        ps_pool = ctx.enter_context(tc.tile_pool(name="ps", bufs=8, space="PSUM"))

        # ---- resident loads ----
        xT = singles.tile([P, JC, BS], bf16)  # x shard, transposed: [jp, jc, b]
        nc.gpsimd.dma_start(
            out=xT, in_=xT_d.ap().rearrange("(jc jp) b -> jp jc b", jp=P)
        )
        # pW rows broadcast to all 128 partitions: [p, m, b]
        pw = singles.tile([P, M, BS], f32)
        pw_src = pw_d.ap()
        pw_bcast = bass.AP(
            tensor=pw_src.tensor, offset=pw_src.offset,
            ap=[[0, P]] + list(pw_src.ap),
        )
        nc.gpsimd.dma_start(out=pw, in_=pw_bcast)
        pbT = singles.tile([M, BS], f32)
        nc.gpsimd.dma_start(out=pbT, in_=pb_d.ap())
        bs_sb = singles.tile([M, DO], f32)
        nc.gpsimd.dma_start(out=bs_sb, in_=bs_d.ap())
        gb = singles.tile([P, BC, DO], bf16)
        nc.gpsimd.dma_start(
            out=gb, in_=gb_d.ap().rearrange("(bc p) i -> p bc i", p=P)
        )
        gw = singles.tile([P, BC, DO], bf16)
        nc.gpsimd.dma_start(
            out=gw, in_=gw_d.ap().rearrange("(bc p) i -> p bc i", p=P)
        )
        z = singles.tile([P, BC, DO], f32)
        eps = singles.tile([P, 1], f32)
        nc.vector.memset(eps, LN_EPS)

        # ---- bias: z = gb * (pb @ bs) ----
        for bc in range(BC):
            for q in range(DO // 512):
                bps = ps_pool.tile([P, 512], f32, tag="acc")
                nc.tensor.matmul(
                    bps,
                    pbT[:, bc * P:(bc + 1) * P],
                    bs_sb[:, q * 512:(q + 1) * 512],
                    start=True, stop=True,
                )
                nc.vector.tensor_mul(
                    z[:, bc, q * 512:(q + 1) * 512], bps,
                    gb[:, bc, q * 512:(q + 1) * 512],
                )

        # ---- main: Wx accumulation over (m, jc), per output half ----
        for ih in range(NIH):
            acc = [[ps_pool.tile([P, 512], f32, tag="acc", name=f"acc_{ih}_{bc}_{q}")
                    for q in range(NQ)] for bc in range(BC)]
            for m in range(M):
                # weights for (m, ih): [jp, jc, i] in two jc-halves
                wst = []
                wsrc = ws_d.ap()[m].rearrange("(jc jp) i -> jp jc i", jp=P)
                for h in range(2):
                    t = ws_pool.tile([P, JC // 2, IH], bf16, tag="ws",
                                     name=f"ws_{ih}_{m}_{h}")
                    nc.sync.dma_start(
                        out=t,
                        in_=wsrc[:, h * (JC // 2):(h + 1) * (JC // 2),
                                 ih * IH:(ih + 1) * IH],
                    )
                    wst.append(t)
                # xs = xT * pW[:, m] (broadcast over jc)
                xs = xs_pool.tile([P, JC, BS], bf16, tag="xs")
                for jc in range(JC):
                    nc.vector.tensor_mul(xs[:, jc, :], xT[:, jc, :], pw[:, m, :])
                for jc in range(JC):
                    w = wst[jc // (JC // 2)]
                    jl = jc % (JC // 2)
                    for bc in range(BC):
                        for q in range(NQ):
                            nc.tensor.matmul(
                                acc[bc][q],
                                xs[:, jc, bc * P:(bc + 1) * P],
                                w[:, jl, q * 512:(q + 1) * 512],
                                start=(m == 0 and jc == 0),
                                stop=(m == M - 1 and jc == JC - 1),
                            )
            # drain: z += acc * gW
            for bc in range(BC):
                for q in range(NQ):
                    i0 = ih * IH + q * 512
                    nc.vector.tensor_mul(
                        acc[bc][q], acc[bc][q], gw[:, bc, i0:i0 + 512]
                    )
                    nc.vector.tensor_add(
                        z[:, bc, i0:i0 + 512], z[:, bc, i0:i0 + 512], acc[bc][q]
                    )

        # ---- LayerNorm + ELU + store, per batch chunk ----
        out_ap = out_d.ap().rearrange("(bc p) i -> p bc i", p=P)
        for bc in range(BC):
            row = z[:, bc, :]
            stats = sm_pool.tile([P, DO // 512, 6], f32, tag="stats")
            for s in range(DO // 512):
                nc.vector.bn_stats(
                    out=stats[:, s, :], in_=row[:, s * 512:(s + 1) * 512]
                )
            mv = sm_pool.tile([P, 2], f32, tag="mv")
            nc.vector.bn_aggr(out=mv, in_=stats)
            rstd = sm_pool.tile([P, 1], f32, tag="rstd")
            nc.scalar.activation(
                out=rstd, in_=mv[:, 1:2], func=AF.Sqrt, bias=eps, scale=1.0
            )
            nc.vector.reciprocal(out=rstd, in_=rstd)
            nmr = sm_pool.tile([P, 1], f32, tag="nmr")
            nc.vector.tensor_mul(nmr, mv[:, 0:1], rstd)
            nc.vector.tensor_scalar_mul(nmr, nmr, -1.0)
            # y = (z - mu) * rstd  (in place)
            nc.scalar.activation(
                out=row, in_=row, func=AF.Identity, bias=nmr, scale=rstd
            )
            # ELU: relu(y) + min(exp(y) - 1, 0)
            et = e_pool.tile([P, DO], f32, tag="et")
            nc.scalar.activation(out=et, in_=row, func=AF.Exp)
            nc.vector.tensor_scalar(
                et, et, -1.0, 0.0, op0=ALU.add, op1=ALU.min
            )
            nc.vector.tensor_scalar_max(row, row, 0.0)
            nc.vector.tensor_add(row, row, et)
            nc.gpsimd.dma_start(out=out_ap[:, bc, :], in_=row)

    nc.compile()
    return nc


def _prep_inputs(x, Ws, bs, pW, pb, gW, gb):
    xT = np.ascontiguousarray(x.astype(BF16).T)          # [DI, B]
    wsT = np.ascontiguousarray(
        Ws.astype(np.float32).transpose(0, 2, 1)
    ).astype(BF16)                                        # [M, DI, DO]
    pwT = np.ascontiguousarray(pW.astype(np.float32).T)  # [M, B]
    pbT = np.ascontiguousarray(pb.astype(np.float32).T)  # [M, B]
    bs = np.ascontiguousarray(bs.astype(np.float32))
    gW16 = gW.astype(BF16)
    gb16 = gb.astype(BF16)
    in_maps = []
    for c in range(NCORES):
        sl = slice(c * BS, (c + 1) * BS)
        in_maps.append({
            "xT": np.ascontiguousarray(xT[:, sl]),
            "wsT": wsT,
            "pwT": np.ascontiguousarray(pwT[:, sl]),
            "pbT": np.ascontiguousarray(pbT[:, sl]),
            "bs": bs,
            "gw": np.ascontiguousarray(gW16[sl]),
            "gb": np.ascontiguousarray(gb16[sl]),
        })
    return in_maps


def kernel(x, Ws, bs, pW, pb, gW, gb, _trace=False, _tmpdir=None):
    from concourse import bass_utils

    if "nc" not in _cache:
        _cache["nc"] = _build()
    nc = _cache["nc"]
    in_maps = _prep_inputs(x, Ws, bs, pW, pb, gW, gb)
    res = bass_utils.run_bass_kernel_spmd(
        nc, in_maps, core_ids=list(range(NCORES)),
        trace=_trace, tmpdir=_tmpdir,
    )
    _cache["last_result"] = res
    out = np.concatenate([res.results[c]["out"] for c in range(NCORES)], axis=0)
    return out.astype(np.float32)
